# revision 19
# baseline (speedup 1.0000x reference)
"""ContextConditionedAttention Trainium2 kernel (v3).

Full-input contract: kernel(**inputs) takes the unsharded numpy inputs and
returns the full (B, N, HIDDEN) float32 output. Work is sharded over 8
NeuronCores as (batch b in 0..3) x (head-group g in 0..1), 4 heads per core.
Each core computes its head-group's partial out-projection (2048, 512); the
host sums the two head-group partials per batch and adds the bias epilogue
(bv @ wo.T + bo, exact because softmax rows sum to 1).

Math simplifications vs the reference (exact):
  - per-(batch,head) softmax bias bias_emb[ct] is constant along the softmax
    axis -> cancels in softmax -> dropped.
  - keymod_emb[ct] adds to K -> folded into the K projection bias.
  - attn_mask folds into the V copy as a per-key 0/1 scale on [V | 1]
    (zeroing both numerator and ones-column denominator of masked keys).
  - 1/sqrt(HEAD_DIM) split as 1/sqrt(8) onto each of Q and K at the
    projection bias-add, so score psum tiles come out pre-scaled.

Engine plan (per core):
  - everything on-chip bf16 except f32 psum/normalize (fp8 scores were
    tried and give ~2e-2 error: quantization noise on q/k acts as a
    correlated perturbation that softmax does not average away).
  - Q^T/K^T tiles [128, 1024]: partitions = head-pair dims (64 | 64),
    tokens free. Scores S^T per (head, key-block): lhsT = K^T slice
    [64, 128], rhs = Q^T slice [64, 512] -> psum [128 keys, 1024 q].
  - exp split: 11/16 key-blocks on ACT (native Exp); 5/16 staged
    psum->sbuf f32 by DVE and computed by the otherwise-idle GPSIMD
    engine as pow(e, s) (vpowf ucode, standard library). Separate p
    tiles per (head-parity, kb) keep the dependency graph loose.
  - PV in O-form: O[q, 65] = sum_kb P_kb^T . [V_kb | 1], queries on
    partitions, 1-bank psum accumulator; denominator in col 64.
    PV for head h is emitted interleaved into head h+1's score stream
    so the in-order PE never waits on exp drain.
  - normalize: DVE reciprocal(col 64) + per-partition multiply into a
    2-head o2 buffer; PE is_transpose matmul (identity rhs, 53ns) flips
    [128 q, 128 d2] -> bf16 psum, DVE 2x copy -> ot_sb. (v4: replaces
    DMA-transpose, whose sem-waits held the SP sequencer ~115us.)
  - out-projection per 128-query block: 2 accumulating matmuls over the
    two 128-dim head-pair chunks, DVE copy psum->sbuf f32, DMA out.
  - all input DMAs on the sync(SP) queue: HWDGE issue is free there,
    while gpsimd-queue DMAs bill ~1us of Pool ENGINE descriptor-gen.

PSUM (8 banks): s0/s1/s2 (2 banks each): score rotation + all
projection tiles; pv0/pv1 (1 bank each): V-proj, PV accumulators,
transpose tiles, and out-proj tiles.
"""

import numpy as np
import ml_dtypes

B, N, HIDDEN = 4, 2048, 512
N_HEADS, HEAD_DIM = 8, 64
G_HEADS = 4          # heads per core (head-group)
G_DIM = 256          # dims per head-group
N_CORES = 8
NKB = N // 128       # key blocks of 128
QC = 1024            # query chunk
SQ = float(1.0 / np.sqrt(np.sqrt(64.0)))   # 1/sqrt(8) folded into Q and K

BF16 = ml_dtypes.bfloat16

# key-blocks whose exp is offloaded to (DVE psum->sbuf copy + Pool pow).
# Early-ish kbs: a pool tile at kb>=14 has ~2.7us latency and gates the
# next slot's PV start.
POOL_KBS = (1, 4, 6, 9, 11, 14)

_CACHE = {}


def _build_program():
    import concourse.bacc as bacc
    import concourse.mybir as mybir
    import concourse.tile as tile

    nc = bacc.Bacc("TRN2", target_bir_lowering=False, debug=False,
                   num_devices=N_CORES)
    f32 = mybir.dt.float32
    bf16 = mybir.dt.bfloat16
    Exp = mybir.ActivationFunctionType.Exp
    Copy = mybir.ActivationFunctionType.Copy
    Pow = mybir.AluOpType.pow
    Add = mybir.AluOpType.add
    Mult = mybir.AluOpType.mult

    # ---- DRAM I/O (per-core shards; same program on all 8 cores) ----
    xt_d = nc.dram_tensor("xt", (HIDDEN, N), bf16, kind="ExternalInput").ap()
    wq_d = nc.dram_tensor("wq", (128, 4, G_DIM), bf16, kind="ExternalInput").ap()
    wk_d = nc.dram_tensor("wk", (128, 4, G_DIM), bf16, kind="ExternalInput").ap()
    wv_d = nc.dram_tensor("wv", (128, 4, G_DIM), bf16, kind="ExternalInput").ap()
    wo_d = nc.dram_tensor("wo", (128, 2, HIDDEN), bf16, kind="ExternalInput").ap()
    qb_d = nc.dram_tensor("qb", (128, 2), f32, kind="ExternalInput").ap()
    kb_d = nc.dram_tensor("kb", (128, 2), f32, kind="ExternalInput").ap()
    mk_d = nc.dram_tensor("mk", (128, NKB), f32, kind="ExternalInput").ap()
    id_d = nc.dram_tensor("id", (128, 128), bf16, kind="ExternalInput").ap()
    y_d = nc.dram_tensor("y", (N, HIDDEN), f32, kind="ExternalOutput").ap()

    with tile.TileContext(nc) as tc:
        with tc.tile_pool(name="sb", bufs=1) as sb, \
             tc.tile_pool(name="ps", bufs=1, space="PSUM") as ps:
            # ---- persistent SBUF tiles ----
            xt_sb = [sb.tile([128, N], bf16, tag=f"xt{c}", name=f"xt{c}")
                     for c in range(4)]
            wq_sb = sb.tile([128, 4, G_DIM], bf16, tag="wq", name="wq_sb")
            wk_sb = sb.tile([128, 4, G_DIM], bf16, tag="wk", name="wk_sb")
            wv_sb = sb.tile([128, 4, G_DIM], bf16, tag="wv", name="wv_sb")
            wo_sb = sb.tile([128, 2, HIDDEN], bf16, tag="wo", name="wo_sb")
            qb_sb = sb.tile([128, 2], f32, tag="qb", name="qb_sb")
            kb_sb = sb.tile([128, 2], f32, tag="kb", name="kb_sb")
            mk_sb = sb.tile([128, NKB], f32, tag="mk", name="mk_sb")
            # Q^T/K^T per (head-pair hp, token chunk tc): [128, 1024] bf16
            qt_sb = [[sb.tile([128, QC], bf16, tag=f"qt{hp}{t}",
                              name=f"qt{hp}{t}") for t in range(2)]
                     for hp in range(2)]
            kt_sb = [[sb.tile([128, QC], bf16, tag=f"kt{hp}{t}",
                              name=f"kt{hp}{t}") for t in range(2)]
                     for hp in range(2)]
            v_sb = sb.tile([128, NKB, G_HEADS, HEAD_DIM + 1], bf16, tag="v",
                           name="v_sb")
            e_sb = sb.tile([128, QC], f32, tag="e", name="e_sb")
            # P tiles per (head-parity, kb): [128 keys, 1024 q] bf16
            p_sb = [[sb.tile([128, QC], bf16, tag=f"p{hp}_{kb}",
                             name=f"p{hp}_{kb}") for kb in range(NKB)]
                    for hp in range(2)]
            stg_sb = [sb.tile([128, QC], f32, tag=f"stg{j}", name=f"stg{j}")
                      for j in range(2)]
            o2_sb = [sb.tile([128, 8, 128], bf16, tag=f"o2{c}", name=f"o2{c}")
                     for c in range(2)]
            ot_sb = sb.tile([128, 2, N], bf16, tag="ot", name="ot_sb")
            rc_sb = [sb.tile([128, 1], f32, tag=f"rc{j}", name=f"rc{j}")
                     for j in range(2)]
            yt_sb = [sb.tile([128, HIDDEN], f32, tag=f"yt{j}", name=f"yt{j}")
                     for j in range(2)]
            id_sb = sb.tile([128, 128], bf16, tag="id", name="id_sb")

            # ---- input DMAs ----
            warm = sb.tile([1, 4], f32, tag="warm", name="warm")
            nc.vector.memset(warm[:], 0.0)
            nc.scalar.activation(warm[:], warm[:], Exp)  # preload Exp table
            nc.sync.dma_start(wk_sb[:], wk_d[:])
            nc.sync.dma_start(wq_sb[:], wq_d[:])
            nc.sync.dma_start(kb_sb[:], kb_d[:])
            nc.sync.dma_start(qb_sb[:], qb_d[:])
            # xt split into per-tcn halves: K00/Q00 only need cols 0:1024,
            # so the first projection matmuls start ~3us earlier.
            for c in range(4):
                nc.sync.dma_start(xt_sb[c][:, 0:QC],
                                  xt_d[c * 128:(c + 1) * 128, 0:QC])
            for c in range(4):
                nc.sync.dma_start(xt_sb[c][:, QC:N],
                                  xt_d[c * 128:(c + 1) * 128, QC:N])
            nc.sync.dma_start(wv_sb[:], wv_d[:])
            nc.sync.dma_start(mk_sb[:], mk_d[:])
            nc.sync.dma_start(wo_sb[:], wo_d[:])
            nc.sync.dma_start(id_sb[:], id_d[:])
            nc.vector.memset(e_sb[:], float(np.e))
            nc.gpsimd.memset(v_sb[:], 1.0)   # ones columns default

            stag = [0]  # rotating psum tag index for s-tags

            def emit_qk(which, hp, tcn):
                """Q/K psum [128, 1024] for head-pair hp, token chunk tcn;
                DVE (psum + bias_hp) * (1/sqrt 8) -> bf16 tile."""
                w_sb, b_sb, dst = ((wq_sb, qb_sb, qt_sb) if which == "q"
                                   else (wk_sb, kb_sb, kt_sb))
                pst = ps.tile([128, QC], f32, tag=f"s{stag[0] % 3}",
                              name=f"ps_{which}{hp}{tcn}")
                stag[0] += 1
                for c in range(4):
                    for s in range(2):
                        fr = tcn * QC + s * 512
                        nc.tensor.matmul(
                            pst[:, s * 512:(s + 1) * 512],
                            lhsT=w_sb[:, c, hp * 128:(hp + 1) * 128],
                            rhs=xt_sb[c][:, fr:fr + 512],
                            start=(c == 0), stop=(c == 3))
                nc.vector.tensor_scalar(
                    dst[hp][tcn][:], pst[:], b_sb[:, hp:hp + 1], SQ, Add, Mult)

            def emit_v(kb):
                """V psum [128, 256] for key-block kb; one masked DVE copy
                into v_sb[:, kb, :, 0:64] (strided 65-per-head)."""
                psv = ps.tile([128, G_DIM], f32, tag=f"pv{kb % 2}",
                              name=f"ps_v{kb}")
                for c in range(4):
                    nc.tensor.matmul(
                        psv[:],
                        lhsT=xt_sb[c][:, kb * 128:(kb + 1) * 128],
                        rhs=wv_sb[:, c, :],
                        start=(c == 0), stop=(c == 3))
                nc.vector.tensor_scalar(
                    v_sb[:, kb, :, 0:HEAD_DIM],
                    psv.rearrange("p (h d) -> p h d", h=G_HEADS),
                    mk_sb[:, kb:kb + 1], None, Mult)

            def emit_scores_kb(qc, h, kb):
                """Score psum [128 keys, 1024 q] + exp/pow -> p_sb."""
                hp = h % 2           # ping-pong parity for p tiles
                pair = h // 2        # head-pair index into qt/kt tiles
                r0 = (h % 2) * 64    # within-pair row offset
                S = ps.tile([128, QC], f32, tag=f"s{stag[0] % 3}",
                            name=f"ps_s{qc}{h}{kb}")
                stag[0] += 1
                for w in range(2):
                    nc.tensor.matmul(
                        S[:, w * 512:(w + 1) * 512],
                        lhsT=kt_sb[pair][kb // 8][r0:r0 + 64,
                                                  (kb % 8) * 128:
                                                  (kb % 8 + 1) * 128],
                        rhs=qt_sb[pair][qc][r0:r0 + 64,
                                            w * 512:(w + 1) * 512],
                        start=True, stop=True)
                dst = p_sb[hp][kb][:]
                if kb in POOL_KBS:
                    stg = stg_sb[kb % 2]
                    nc.vector.tensor_copy(stg[:], S[:])
                    nc.gpsimd.tensor_tensor(dst, e_sb[:], stg[:], Pow)
                else:
                    nc.scalar.activation(dst, S[:], Exp)

            def emit_pv_qb(qc, h, qb, tag=None):
                """O-form PV for one 128-query block + normalize into o2."""
                hp = h % 2
                O = ps.tile([128, HEAD_DIM + 1], f32, tag=tag or f"pv{qb % 2}",
                            name=f"ps_o{qc}{h}{qb}")
                for kb in range(NKB):
                    nc.tensor.matmul(
                        O[:],
                        lhsT=p_sb[hp][kb][:, qb * 128:(qb + 1) * 128],
                        rhs=v_sb[:, kb, h, :],
                        start=(kb == 0), stop=(kb == NKB - 1))
                rc = rc_sb[qb % 2]
                nc.vector.reciprocal(rc[:], O[:, 64:65])
                nc.vector.tensor_scalar(
                    o2_sb[h // 2][:, qb, hp * 64:hp * 64 + 64],
                    O[:, 0:HEAD_DIM], rc[:, 0:1], None, Mult)

            def emit_transpose(qc, c, qb, tag=None):
                tp = ps.tile([128, 128], bf16, tag=tag or f"pv{qb % 2}",
                             name=f"ps_t{qc}{c}{qb}")
                nc.tensor.transpose(tp[:], o2_sb[c][:, qb, :], id_sb[:])
                nc.vector.tensor_copy(
                    ot_sb[:, c, qc * QC + qb * 128:qc * QC + (qb + 1) * 128],
                    tp[:])

            def emit_out_proj_qb(qc, qb):
                q0 = qc * QC + qb * 128
                Y = ps.tile([128, HIDDEN], f32, tag=f"pv{qb % 2}",
                            name=f"ps_y{qc}{qb}")
                for c in range(2):
                    nc.tensor.matmul(
                        Y[:],
                        lhsT=ot_sb[:, c, q0:q0 + 128],
                        rhs=wo_sb[:, c, :],
                        start=(c == 0), stop=(c == 1))
                yt = yt_sb[qb % 2]
                if qc == 1:
                    # tail: ACT is exp-idle by now, DVE is the tail pacer
                    nc.scalar.activation(yt[:], Y[:], Copy)
                else:
                    nc.vector.tensor_copy(yt[:], Y[:])
                nc.sync.dma_start(y_d[q0:q0 + 128, :], yt[:])

            # ---- PE warm-up: dep-free junk transposes keep the PE busy
            # through the input-DMA wait so K00 starts at full p-state
            # (cold matmuls run 2-4x slower until ~3us of continuous
            # execution). Results are garbage and never read. ----
            jk_sb = sb.tile([128, 128], bf16, tag="jk", name="jk_sb")
            nc.vector.memset(jk_sb[:], 0.0)
            for j in range(48):
                jps = ps.tile([128, 128], bf16, tag=f"pv{j % 2}",
                              name=f"ps_j{j}")
                nc.tensor.transpose(jps[:], jk_sb[:], jk_sb[:])

            # ---- lead-in: just h0's Q/K. V-projection is interleaved into
            # slots 0-1 (PV of h0 first reads V in slot 1). ----
            emit_qk("k", 0, 0)
            emit_qk("q", 0, 0)

            # deferred projection work, interleaved into the attention
            # stream at (slot, kb) chosen to even out PE load per slot
            # while meeting need-bys: K01 by s0kb8; K10/K11 by s2 scores
            # lhsT; Q10 by s2kb0 rhs; Q01 by s4; Q11 by s6.
            deferred = {
                (0, 3): lambda: emit_qk("k", 0, 1),
                (1, 3): lambda: emit_qk("k", 1, 0),
                (1, 11): lambda: emit_qk("q", 1, 0),
                (2, 3): lambda: emit_qk("k", 1, 1),
                (2, 11): lambda: emit_qk("q", 0, 1),
                (3, 11): lambda: emit_qk("q", 1, 1),
            }
            vq = list(range(NKB))   # V-projection kbs, drained slots 0-1
            post = []   # pending per-head PV/transpose/out-proj closures

            def emit_slot(qc, h, si):
                """One head-slot: scores+exp for (qc, h) interleaved with
                pending PV of the previous head, V-projection (slots 0-1)
                and deferred projections. Posts pop at kb>=3 so the
                previous slot's last exps (incl. pool-path latency) are
                done before the in-order PE hits the PV matmuls that read
                them. post MUST fully drain within the slot: the p tiles
                it reads are double-buffered and rewritten next slot."""
                for kb in range(NKB):
                    emit_scores_kb(qc, h, kb)
                    if vq and (si == 0 and kb < 14 or si == 1 and kb < 2):
                        emit_v(vq.pop(0))
                    if kb >= 3 and post:
                        post.pop(0)()
                    d = deferred.pop((si, kb), None)
                    if d:
                        d()
                while post:
                    post.pop(0)()

            def make_post(qc, h, qb, tail=False):
                # in the tail (after the last score slot) the s-banks are
                # free: spread PV/transpose psums over them so the per-qb
                # finalize chains aren't serialized on two pv tags.
                ptag = f"s{qb % 3}" if tail else None
                def f():
                    emit_pv_qb(qc, h, qb, tag=ptag)
                    if h % 2 == 1:
                        emit_transpose(qc, h // 2, qb, tag=ptag)
                    if h == 3:
                        emit_out_proj_qb(qc, qb)
                return f

            slots = [(qc, h) for qc in range(2) for h in range(G_HEADS)]
            for i, (qc, h) in enumerate(slots):
                emit_slot(qc, h, i)
                # queue this head's PV (+ per-qb transpose/out-proj chain)
                for qb in range(8):
                    post.append(make_post(qc, h, qb, tail=(i == 7)))

            # drain everything still pending
            while post:
                post.pop(0)()
            while deferred:
                deferred.pop(0)()

    nc.compile()
    return nc


def _get_program():
    if "nc" not in _CACHE:
        _CACHE["nc"] = _build_program()
    return _CACHE["nc"]


def _prep_inputs(x, cancer_type, attn_mask, wq, bq, wk, bk, wv, bv, wo, bo,
                 bias_emb, keymod_emb):
    """Host-side shard prep: returns (in_maps list of 8, epilogue (512,))."""
    x = np.asarray(x, dtype=np.float32)
    ct = np.asarray(cancer_type).astype(np.int64)
    mask = np.asarray(attn_mask)
    wq = np.asarray(wq, dtype=np.float32)
    wk = np.asarray(wk, dtype=np.float32)
    wv = np.asarray(wv, dtype=np.float32)
    wo = np.asarray(wo, dtype=np.float32)
    bq = np.asarray(bq, dtype=np.float32)
    bk = np.asarray(bk, dtype=np.float32)
    bv = np.asarray(bv, dtype=np.float32)
    bo = np.asarray(bo, dtype=np.float32)
    keymod = np.asarray(keymod_emb, dtype=np.float32)

    wqt = np.ascontiguousarray(wq.T).astype(BF16)     # (in 512, out 512)
    wkt = np.ascontiguousarray(wk.T).astype(BF16)
    wvt = np.ascontiguousarray(wv.T).astype(BF16)
    wot = np.ascontiguousarray(wo.T).astype(BF16)

    xt_all = [np.ascontiguousarray(x[b].T).astype(BF16) for b in range(B)]
    mks = [np.where(mask[b], np.float32(0.0), np.float32(1.0))
           .reshape(NKB, 128).T.copy() for b in range(B)]

    in_maps = []
    for core in range(N_CORES):
        b, g = core // 2, core % 2
        gs = slice(g * G_DIM, (g + 1) * G_DIM)
        qbias = np.ascontiguousarray(
            (bq[gs] * SQ).reshape(2, 128).T).astype(np.float32)
        kfull = (bk + keymod[ct[b]])[gs] * SQ
        kbias = np.ascontiguousarray(kfull.reshape(2, 128).T).astype(np.float32)
        in_maps.append({
            "id": np.eye(128, dtype=BF16),
            "xt": xt_all[b],
            "wq": np.ascontiguousarray(
                wqt[:, gs].reshape(4, 128, G_DIM).transpose(1, 0, 2)),
            "wk": np.ascontiguousarray(
                wkt[:, gs].reshape(4, 128, G_DIM).transpose(1, 0, 2)),
            "wv": np.ascontiguousarray(
                wvt[:, gs].reshape(4, 128, G_DIM).transpose(1, 0, 2)),
            "wo": np.ascontiguousarray(
                wot[gs, :].reshape(2, 128, HIDDEN).transpose(1, 0, 2)),
            "qb": qbias,
            "kb": kbias,
            "mk": mks[b],
        })
    epilogue = (bv @ wo.T + bo).astype(np.float32)    # (512,)
    return in_maps, epilogue


def kernel(**inputs):
    from concourse import bass_utils

    nc = _get_program()
    in_maps, epilogue = _prep_inputs(**inputs)
    res = bass_utils.run_bass_kernel_spmd(nc, in_maps,
                                          core_ids=list(range(N_CORES)))
    out = np.empty((B, N, HIDDEN), dtype=np.float32)
    for b in range(B):
        out[b] = res.results[2 * b]["y"] + res.results[2 * b + 1]["y"] + epilogue
    return out



# revision 20
# speedup vs baseline: 1.0797x; 1.0797x over previous
"""ContextConditionedAttention Trainium2 kernel (v3).

Full-input contract: kernel(**inputs) takes the unsharded numpy inputs and
returns the full (B, N, HIDDEN) float32 output. Work is sharded over 8
NeuronCores as (batch b in 0..3) x (head-group g in 0..1), 4 heads per core.
Each core computes its head-group's partial out-projection (2048, 512); the
host sums the two head-group partials per batch and adds the bias epilogue
(bv @ wo.T + bo, exact because softmax rows sum to 1).

Math simplifications vs the reference (exact):
  - per-(batch,head) softmax bias bias_emb[ct] is constant along the softmax
    axis -> cancels in softmax -> dropped.
  - keymod_emb[ct] adds to K -> folded into the K projection bias.
  - attn_mask folds into the V copy as a per-key 0/1 scale on [V | 1]
    (zeroing both numerator and ones-column denominator of masked keys).
  - 1/sqrt(HEAD_DIM) split as 1/sqrt(8) onto each of Q and K at the
    projection bias-add, so score psum tiles come out pre-scaled.

Engine plan (per core):
  - everything on-chip bf16 except f32 psum/normalize (fp8 scores were
    tried and give ~2e-2 error: quantization noise on q/k acts as a
    correlated perturbation that softmax does not average away).
  - Q^T/K^T tiles [128, 1024]: partitions = head-pair dims (64 | 64),
    tokens free. Scores S^T per (head, key-block): lhsT = K^T slice
    [64, 128], rhs = Q^T slice [64, 512] -> psum [128 keys, 1024 q].
  - exp split: 11/16 key-blocks on ACT (native Exp); 5/16 staged
    psum->sbuf f32 by DVE and computed by the otherwise-idle GPSIMD
    engine as pow(e, s) (vpowf ucode, standard library). Separate p
    tiles per (head-parity, kb) keep the dependency graph loose.
  - PV in O-form: O[q, 65] = sum_kb P_kb^T . [V_kb | 1], queries on
    partitions, 1-bank psum accumulator; denominator in col 64.
    PV for head h is emitted interleaved into head h+1's score stream
    so the in-order PE never waits on exp drain.
  - normalize: DVE reciprocal(col 64) + per-partition multiply into a
    2-head o2 buffer; PE is_transpose matmul (identity rhs, 53ns) flips
    [128 q, 128 d2] -> bf16 psum, DVE 2x copy -> ot_sb. (v4: replaces
    DMA-transpose, whose sem-waits held the SP sequencer ~115us.)
  - out-projection per 128-query block: 2 accumulating matmuls over the
    two 128-dim head-pair chunks, DVE copy psum->sbuf f32, DMA out.
  - all input DMAs on the sync(SP) queue: HWDGE issue is free there,
    while gpsimd-queue DMAs bill ~1us of Pool ENGINE descriptor-gen.

PSUM (8 banks): s0/s1/s2 (2 banks each): score rotation + all
projection tiles; pv0/pv1 (1 bank each): V-proj, PV accumulators,
transpose tiles, and out-proj tiles.
"""

import numpy as np
import ml_dtypes

B, N, HIDDEN = 4, 2048, 512
N_HEADS, HEAD_DIM = 8, 64
G_HEADS = 4          # heads per core (head-group)
G_DIM = 256          # dims per head-group
N_CORES = 8
NKB = N // 128       # key blocks of 128
QC = 1024            # query chunk
SQ = float(1.0 / np.sqrt(np.sqrt(64.0)))   # 1/sqrt(8) folded into Q and K

BF16 = ml_dtypes.bfloat16

# key-blocks whose exp is offloaded to (DVE psum->sbuf copy + Pool pow).
# Early-ish kbs: a pool tile at kb>=14 has ~2.7us latency and gates the
# next slot's PV start.
POOL_KBS = (1, 4, 7, 10, 13)

_CACHE = {}


def _build_program():
    import concourse.bacc as bacc
    import concourse.mybir as mybir
    import concourse.tile as tile

    nc = bacc.Bacc("TRN2", target_bir_lowering=False, debug=False,
                   num_devices=N_CORES)
    f32 = mybir.dt.float32
    bf16 = mybir.dt.bfloat16
    Exp = mybir.ActivationFunctionType.Exp
    Copy = mybir.ActivationFunctionType.Copy
    Pow = mybir.AluOpType.pow
    Add = mybir.AluOpType.add
    Mult = mybir.AluOpType.mult

    # ---- DRAM I/O (per-core shards; same program on all 8 cores) ----
    xt_d = nc.dram_tensor("xt", (HIDDEN, N), bf16, kind="ExternalInput").ap()
    wq_d = nc.dram_tensor("wq", (128, 4, G_DIM), bf16, kind="ExternalInput").ap()
    wk_d = nc.dram_tensor("wk", (128, 4, G_DIM), bf16, kind="ExternalInput").ap()
    wv_d = nc.dram_tensor("wv", (128, 4, G_DIM), bf16, kind="ExternalInput").ap()
    wo_d = nc.dram_tensor("wo", (128, 2, HIDDEN), bf16, kind="ExternalInput").ap()
    qb_d = nc.dram_tensor("qb", (128, 2), f32, kind="ExternalInput").ap()
    kb_d = nc.dram_tensor("kb", (128, 2), f32, kind="ExternalInput").ap()
    mk_d = nc.dram_tensor("mk", (128, NKB), f32, kind="ExternalInput").ap()
    id_d = nc.dram_tensor("id", (128, 128), bf16, kind="ExternalInput").ap()
    y_d = nc.dram_tensor("y", (N, HIDDEN), f32, kind="ExternalOutput").ap()

    with tile.TileContext(nc) as tc:
        with tc.tile_pool(name="sb", bufs=1) as sb, \
             tc.tile_pool(name="ps", bufs=1, space="PSUM") as ps:
            # ---- persistent SBUF tiles ----
            xt_sb = [sb.tile([128, N], bf16, tag=f"xt{c}", name=f"xt{c}")
                     for c in range(4)]
            wq_sb = sb.tile([128, 4, G_DIM], bf16, tag="wq", name="wq_sb")
            wk_sb = sb.tile([128, 4, G_DIM], bf16, tag="wk", name="wk_sb")
            wv_sb = sb.tile([128, 4, G_DIM], bf16, tag="wv", name="wv_sb")
            wo_sb = sb.tile([128, 2, HIDDEN], bf16, tag="wo", name="wo_sb")
            qb_sb = sb.tile([128, 2], f32, tag="qb", name="qb_sb")
            kb_sb = sb.tile([128, 2], f32, tag="kb", name="kb_sb")
            mk_sb = sb.tile([128, NKB], f32, tag="mk", name="mk_sb")
            # Q^T/K^T per (head-pair hp, token chunk tc): [128, 1024] bf16
            qt_sb = [[sb.tile([128, QC], bf16, tag=f"qt{hp}{t}",
                              name=f"qt{hp}{t}") for t in range(2)]
                     for hp in range(2)]
            kt_sb = [[sb.tile([128, QC], bf16, tag=f"kt{hp}{t}",
                              name=f"kt{hp}{t}") for t in range(2)]
                     for hp in range(2)]
            v_sb = sb.tile([128, NKB, G_HEADS, HEAD_DIM + 1], bf16, tag="v",
                           name="v_sb")
            e_sb = sb.tile([128, QC], f32, tag="e", name="e_sb")
            # P tiles per (head-parity, kb): [128 keys, 1024 q] bf16
            p_sb = [[sb.tile([128, QC], bf16, tag=f"p{hp}_{kb}",
                             name=f"p{hp}_{kb}") for kb in range(NKB)]
                    for hp in range(2)]
            stg_sb = [sb.tile([128, QC], f32, tag=f"stg{j}", name=f"stg{j}")
                      for j in range(2)]
            o2_sb = [sb.tile([128, 8, 128], bf16, tag=f"o2{c}", name=f"o2{c}")
                     for c in range(2)]
            ot_sb = sb.tile([128, 2, N], bf16, tag="ot", name="ot_sb")
            rc_sb = [sb.tile([128, 1], f32, tag=f"rc{j}", name=f"rc{j}")
                     for j in range(2)]
            yt_sb = [sb.tile([128, HIDDEN], f32, tag=f"yt{j}", name=f"yt{j}")
                     for j in range(2)]
            id_sb = sb.tile([128, 128], bf16, tag="id", name="id_sb")

            # ---- input DMAs ----
            warm = sb.tile([1, 4], f32, tag="warm", name="warm")
            nc.vector.memset(warm[:], 0.0)
            nc.scalar.activation(warm[:], warm[:], Exp)  # preload Exp table
            nc.sync.dma_start(wk_sb[:], wk_d[:])
            nc.sync.dma_start(wq_sb[:], wq_d[:])
            nc.sync.dma_start(kb_sb[:], kb_d[:])
            nc.sync.dma_start(qb_sb[:], qb_d[:])
            # xt split into per-tcn halves: K00/Q00 only need cols 0:1024,
            # so the first projection matmuls start ~3us earlier.
            for c in range(4):
                nc.sync.dma_start(xt_sb[c][:, 0:QC],
                                  xt_d[c * 128:(c + 1) * 128, 0:QC])
            for c in range(4):
                nc.sync.dma_start(xt_sb[c][:, QC:N],
                                  xt_d[c * 128:(c + 1) * 128, QC:N])
            nc.sync.dma_start(wv_sb[:], wv_d[:])
            nc.sync.dma_start(mk_sb[:], mk_d[:])
            nc.sync.dma_start(wo_sb[:], wo_d[:])
            nc.sync.dma_start(id_sb[:], id_d[:])
            nc.vector.memset(e_sb[:], float(np.e))
            nc.gpsimd.memset(v_sb[:], 1.0)   # ones columns default

            stag = [0]  # rotating psum tag index for s-tags

            def emit_qk(which, hp, tcn):
                """Q/K psum [128, 1024] for head-pair hp, token chunk tcn;
                DVE (psum + bias_hp) * (1/sqrt 8) -> bf16 tile."""
                w_sb, b_sb, dst = ((wq_sb, qb_sb, qt_sb) if which == "q"
                                   else (wk_sb, kb_sb, kt_sb))
                pst = ps.tile([128, QC], f32, tag=f"s{stag[0] % 3}",
                              name=f"ps_{which}{hp}{tcn}")
                stag[0] += 1
                for c in range(4):
                    for s in range(2):
                        fr = tcn * QC + s * 512
                        nc.tensor.matmul(
                            pst[:, s * 512:(s + 1) * 512],
                            lhsT=w_sb[:, c, hp * 128:(hp + 1) * 128],
                            rhs=xt_sb[c][:, fr:fr + 512],
                            start=(c == 0), stop=(c == 3))
                nc.vector.tensor_scalar(
                    dst[hp][tcn][:], pst[:], b_sb[:, hp:hp + 1], SQ, Add, Mult)

            def emit_v(kb):
                """V psum [128, 256] for key-block kb; one masked DVE copy
                into v_sb[:, kb, :, 0:64] (strided 65-per-head)."""
                psv = ps.tile([128, G_DIM], f32, tag=f"pv{kb % 2}",
                              name=f"ps_v{kb}")
                for c in range(4):
                    nc.tensor.matmul(
                        psv[:],
                        lhsT=xt_sb[c][:, kb * 128:(kb + 1) * 128],
                        rhs=wv_sb[:, c, :],
                        start=(c == 0), stop=(c == 3))
                nc.vector.tensor_scalar(
                    v_sb[:, kb, :, 0:HEAD_DIM],
                    psv.rearrange("p (h d) -> p h d", h=G_HEADS),
                    mk_sb[:, kb:kb + 1], None, Mult)

            def emit_scores_kb(qc, h, kb):
                """Score psum [128 keys, 1024 q] + exp/pow -> p_sb."""
                hp = h % 2           # ping-pong parity for p tiles
                pair = h // 2        # head-pair index into qt/kt tiles
                r0 = (h % 2) * 64    # within-pair row offset
                S = ps.tile([128, QC], f32, tag=f"s{stag[0] % 3}",
                            name=f"ps_s{qc}{h}{kb}")
                stag[0] += 1
                for w in range(2):
                    nc.tensor.matmul(
                        S[:, w * 512:(w + 1) * 512],
                        lhsT=kt_sb[pair][kb // 8][r0:r0 + 64,
                                                  (kb % 8) * 128:
                                                  (kb % 8 + 1) * 128],
                        rhs=qt_sb[pair][qc][r0:r0 + 64,
                                            w * 512:(w + 1) * 512],
                        start=True, stop=True)
                dst = p_sb[hp][kb][:]
                if kb in POOL_KBS:
                    stg = stg_sb[kb % 2]
                    nc.vector.tensor_copy(stg[:], S[:])
                    nc.gpsimd.tensor_tensor(dst, e_sb[:], stg[:], Pow)
                else:
                    nc.scalar.activation(dst, S[:], Exp)

            def emit_pv_qb(qc, h, qb, tag=None):
                """O-form PV for one 128-query block + normalize into o2."""
                hp = h % 2
                O = ps.tile([128, HEAD_DIM + 1], f32, tag=tag or f"pv{qb % 2}",
                            name=f"ps_o{qc}{h}{qb}")
                for kb in range(NKB):
                    nc.tensor.matmul(
                        O[:],
                        lhsT=p_sb[hp][kb][:, qb * 128:(qb + 1) * 128],
                        rhs=v_sb[:, kb, h, :],
                        start=(kb == 0), stop=(kb == NKB - 1))
                rc = rc_sb[qb % 2]
                nc.vector.reciprocal(rc[:], O[:, 64:65])
                nc.vector.tensor_scalar(
                    o2_sb[h // 2][:, qb, hp * 64:hp * 64 + 64],
                    O[:, 0:HEAD_DIM], rc[:, 0:1], None, Mult)

            def emit_transpose(qc, c, qb, tag=None):
                tp = ps.tile([128, 128], bf16, tag=tag or f"pv{qb % 2}",
                             name=f"ps_t{qc}{c}{qb}")
                nc.tensor.transpose(tp[:], o2_sb[c][:, qb, :], id_sb[:])
                nc.vector.tensor_copy(
                    ot_sb[:, c, qc * QC + qb * 128:qc * QC + (qb + 1) * 128],
                    tp[:])

            def emit_out_proj_qb(qc, qb):
                q0 = qc * QC + qb * 128
                Y = ps.tile([128, HIDDEN], f32, tag=f"pv{qb % 2}",
                            name=f"ps_y{qc}{qb}")
                for c in range(2):
                    nc.tensor.matmul(
                        Y[:],
                        lhsT=ot_sb[:, c, q0:q0 + 128],
                        rhs=wo_sb[:, c, :],
                        start=(c == 0), stop=(c == 1))
                yt = yt_sb[qb % 2]
                if qc == 1:
                    # tail: ACT is exp-idle by now, DVE is the tail pacer
                    nc.scalar.activation(yt[:], Y[:], Copy)
                else:
                    nc.vector.tensor_copy(yt[:], Y[:])
                nc.sync.dma_start(y_d[q0:q0 + 128, :], yt[:])

            # ---- PE warm-up: dep-free junk transposes keep the PE busy
            # through the input-DMA wait so K00 starts at full p-state
            # (cold matmuls run 2-4x slower until ~3us of continuous
            # execution). Results are garbage and never read. ----
            jk_sb = sb.tile([128, 128], bf16, tag="jk", name="jk_sb")
            nc.vector.memset(jk_sb[:], 0.0)
            for j in range(48):
                jps = ps.tile([128, 128], bf16, tag=f"pv{j % 2}",
                              name=f"ps_j{j}")
                nc.tensor.transpose(jps[:], jk_sb[:], jk_sb[:])

            # ---- lead-in: just h0's Q/K. V-projection is interleaved into
            # slots 0-1 (PV of h0 first reads V in slot 1). ----
            emit_qk("k", 0, 0)
            emit_qk("q", 0, 0)

            # deferred projection work, interleaved into the attention
            # stream at (slot, kb) chosen to even out PE load per slot
            # while meeting need-bys: K01 by s0kb8; K10/K11 by s2 scores
            # lhsT; Q10 by s2kb0 rhs; Q01 by s4; Q11 by s6.
            deferred = {
                (0, 3): lambda: emit_qk("k", 0, 1),
                (1, 3): lambda: emit_qk("k", 1, 0),
                (1, 11): lambda: emit_qk("q", 1, 0),
                (2, 3): lambda: emit_qk("k", 1, 1),
                (2, 11): lambda: emit_qk("q", 0, 1),
                (3, 11): lambda: emit_qk("q", 1, 1),
            }
            vq = list(range(NKB))   # V-projection kbs, drained slots 0-1
            post = []   # pending per-head PV/transpose/out-proj closures

            def emit_slot(qc, h, si):
                """One head-slot: scores+exp for (qc, h) interleaved with
                pending PV of the previous head, V-projection (slots 0-1)
                and deferred projections. Posts pop at kb>=3 so the
                previous slot's last exps (incl. pool-path latency) are
                done before the in-order PE hits the PV matmuls that read
                them. post MUST fully drain within the slot: the p tiles
                it reads are double-buffered and rewritten next slot."""
                for kb in range(NKB):
                    emit_scores_kb(qc, h, kb)
                    if vq and (si == 0 and kb < 14 or si == 1 and kb < 2):
                        emit_v(vq.pop(0))
                    if kb >= 3 and post:
                        post.pop(0)()
                    d = deferred.pop((si, kb), None)
                    if d:
                        d()
                while post:
                    post.pop(0)()

            def make_post(qc, h, qb, tail=False):
                # in the tail (after the last score slot) the s-banks are
                # free: spread PV/transpose psums over them so the per-qb
                # finalize chains aren't serialized on two pv tags.
                ptag = f"s{qb % 3}" if tail else None
                def f():
                    emit_pv_qb(qc, h, qb, tag=ptag)
                    if h % 2 == 1:
                        emit_transpose(qc, h // 2, qb, tag=ptag)
                    if h == 3:
                        emit_out_proj_qb(qc, qb)
                return f

            slots = [(qc, h) for qc in range(2) for h in range(G_HEADS)]
            for i, (qc, h) in enumerate(slots):
                emit_slot(qc, h, i)
                # queue this head's PV (+ per-qb transpose/out-proj chain)
                for qb in range(8):
                    post.append(make_post(qc, h, qb, tail=(i == 7)))

            # drain everything still pending
            while post:
                post.pop(0)()
            while deferred:
                deferred.pop(0)()

    nc.compile()
    return nc


def _get_program():
    if "nc" not in _CACHE:
        _CACHE["nc"] = _build_program()
    return _CACHE["nc"]


def _prep_inputs(x, cancer_type, attn_mask, wq, bq, wk, bk, wv, bv, wo, bo,
                 bias_emb, keymod_emb):
    """Host-side shard prep: returns (in_maps list of 8, epilogue (512,))."""
    x = np.asarray(x, dtype=np.float32)
    ct = np.asarray(cancer_type).astype(np.int64)
    mask = np.asarray(attn_mask)
    wq = np.asarray(wq, dtype=np.float32)
    wk = np.asarray(wk, dtype=np.float32)
    wv = np.asarray(wv, dtype=np.float32)
    wo = np.asarray(wo, dtype=np.float32)
    bq = np.asarray(bq, dtype=np.float32)
    bk = np.asarray(bk, dtype=np.float32)
    bv = np.asarray(bv, dtype=np.float32)
    bo = np.asarray(bo, dtype=np.float32)
    keymod = np.asarray(keymod_emb, dtype=np.float32)

    wqt = np.ascontiguousarray(wq.T).astype(BF16)     # (in 512, out 512)
    wkt = np.ascontiguousarray(wk.T).astype(BF16)
    wvt = np.ascontiguousarray(wv.T).astype(BF16)
    wot = np.ascontiguousarray(wo.T).astype(BF16)

    xt_all = [np.ascontiguousarray(x[b].T).astype(BF16) for b in range(B)]
    mks = [np.where(mask[b], np.float32(0.0), np.float32(1.0))
           .reshape(NKB, 128).T.copy() for b in range(B)]

    in_maps = []
    for core in range(N_CORES):
        b, g = core // 2, core % 2
        gs = slice(g * G_DIM, (g + 1) * G_DIM)
        qbias = np.ascontiguousarray(
            (bq[gs] * SQ).reshape(2, 128).T).astype(np.float32)
        kfull = (bk + keymod[ct[b]])[gs] * SQ
        kbias = np.ascontiguousarray(kfull.reshape(2, 128).T).astype(np.float32)
        in_maps.append({
            "id": np.eye(128, dtype=BF16),
            "xt": xt_all[b],
            "wq": np.ascontiguousarray(
                wqt[:, gs].reshape(4, 128, G_DIM).transpose(1, 0, 2)),
            "wk": np.ascontiguousarray(
                wkt[:, gs].reshape(4, 128, G_DIM).transpose(1, 0, 2)),
            "wv": np.ascontiguousarray(
                wvt[:, gs].reshape(4, 128, G_DIM).transpose(1, 0, 2)),
            "wo": np.ascontiguousarray(
                wot[gs, :].reshape(2, 128, HIDDEN).transpose(1, 0, 2)),
            "qb": qbias,
            "kb": kbias,
            "mk": mks[b],
        })
    epilogue = (bv @ wo.T + bo).astype(np.float32)    # (512,)
    return in_maps, epilogue


def kernel(**inputs):
    from concourse import bass_utils

    nc = _get_program()
    in_maps, epilogue = _prep_inputs(**inputs)
    res = bass_utils.run_bass_kernel_spmd(nc, in_maps,
                                          core_ids=list(range(N_CORES)))
    out = np.empty((B, N, HIDDEN), dtype=np.float32)
    for b in range(B):
        out[b] = res.results[2 * b]["y"] + res.results[2 * b + 1]["y"] + epilogue
    return out



# revision 22
# speedup vs baseline: 1.0906x; 1.0101x over previous
"""ContextConditionedAttention Trainium2 kernel (v3).

Full-input contract: kernel(**inputs) takes the unsharded numpy inputs and
returns the full (B, N, HIDDEN) float32 output. Work is sharded over 8
NeuronCores as (batch b in 0..3) x (head-group g in 0..1), 4 heads per core.
Each core computes its head-group's partial out-projection (2048, 512); the
host sums the two head-group partials per batch and adds the bias epilogue
(bv @ wo.T + bo, exact because softmax rows sum to 1).

Math simplifications vs the reference (exact):
  - per-(batch,head) softmax bias bias_emb[ct] is constant along the softmax
    axis -> cancels in softmax -> dropped.
  - keymod_emb[ct] adds to K -> folded into the K projection bias.
  - attn_mask folds into the V copy as a per-key 0/1 scale on [V | 1]
    (zeroing both numerator and ones-column denominator of masked keys).
  - 1/sqrt(HEAD_DIM) split as 1/sqrt(8) onto each of Q and K at the
    projection bias-add, so score psum tiles come out pre-scaled.

Engine plan (per core):
  - everything on-chip bf16 except f32 psum/normalize (fp8 scores were
    tried and give ~2e-2 error: quantization noise on q/k acts as a
    correlated perturbation that softmax does not average away).
  - Q^T/K^T tiles [128, 1024]: partitions = head-pair dims (64 | 64),
    tokens free. Scores S^T per (head, key-block): lhsT = K^T slice
    [64, 128], rhs = Q^T slice [64, 512] -> psum [128 keys, 1024 q].
  - exp split: 11/16 key-blocks on ACT (native Exp); 5/16 staged
    psum->sbuf f32 by DVE and computed by the otherwise-idle GPSIMD
    engine as pow(e, s) (vpowf ucode, standard library). Separate p
    tiles per (head-parity, kb) keep the dependency graph loose.
  - PV in O-form: O[q, 65] = sum_kb P_kb^T . [V_kb | 1], queries on
    partitions, 1-bank psum accumulator; denominator in col 64.
    PV for head h is emitted interleaved into head h+1's score stream
    so the in-order PE never waits on exp drain.
  - normalize: DVE reciprocal(col 64) + per-partition multiply into a
    2-head o2 buffer; PE is_transpose matmul (identity rhs, 53ns) flips
    [128 q, 128 d2] -> bf16 psum, DVE 2x copy -> ot_sb. (v4: replaces
    DMA-transpose, whose sem-waits held the SP sequencer ~115us.)
  - out-projection per 128-query block: 2 accumulating matmuls over the
    two 128-dim head-pair chunks, DVE copy psum->sbuf f32, DMA out.
  - all input DMAs on the sync(SP) queue: HWDGE issue is free there,
    while gpsimd-queue DMAs bill ~1us of Pool ENGINE descriptor-gen.

PSUM (8 banks): s0/s1/s2 (2 banks each): score rotation + all
projection tiles; pv0/pv1 (1 bank each): V-proj, PV accumulators,
transpose tiles, and out-proj tiles.
"""

import numpy as np
import ml_dtypes

B, N, HIDDEN = 4, 2048, 512
N_HEADS, HEAD_DIM = 8, 64
G_HEADS = 4          # heads per core (head-group)
G_DIM = 256          # dims per head-group
N_CORES = 8
NKB = N // 128       # key blocks of 128
QC = 1024            # query chunk
SQ = float(1.0 / np.sqrt(np.sqrt(64.0)))   # 1/sqrt(8) folded into Q and K

BF16 = ml_dtypes.bfloat16

# key-blocks whose exp is offloaded to (DVE psum->sbuf copy + Pool pow).
# Early-ish kbs: a pool tile at kb>=14 has ~2.7us latency and gates the
# next slot's PV start.
POOL_KBS = (1, 4, 7, 10, 13)

_CACHE = {}


def _build_program():
    import concourse.bacc as bacc
    import concourse.mybir as mybir
    import concourse.tile as tile

    nc = bacc.Bacc("TRN2", target_bir_lowering=False, debug=False,
                   num_devices=N_CORES)
    f32 = mybir.dt.float32
    bf16 = mybir.dt.bfloat16
    Exp = mybir.ActivationFunctionType.Exp
    Copy = mybir.ActivationFunctionType.Copy
    Pow = mybir.AluOpType.pow
    Add = mybir.AluOpType.add
    Mult = mybir.AluOpType.mult

    # ---- DRAM I/O (per-core shards; same program on all 8 cores) ----
    xt_d = nc.dram_tensor("xt", (HIDDEN, N), bf16, kind="ExternalInput").ap()
    wq_d = nc.dram_tensor("wq", (128, 4, G_DIM), bf16, kind="ExternalInput").ap()
    wk_d = nc.dram_tensor("wk", (128, 4, G_DIM), bf16, kind="ExternalInput").ap()
    wv_d = nc.dram_tensor("wv", (128, 4, G_DIM), bf16, kind="ExternalInput").ap()
    wo_d = nc.dram_tensor("wo", (128, 2, HIDDEN), bf16, kind="ExternalInput").ap()
    qb_d = nc.dram_tensor("qb", (128, 2), f32, kind="ExternalInput").ap()
    kb_d = nc.dram_tensor("kb", (128, 2), f32, kind="ExternalInput").ap()
    mk_d = nc.dram_tensor("mk", (128, NKB), f32, kind="ExternalInput").ap()
    id_d = nc.dram_tensor("id", (128, 128), bf16, kind="ExternalInput").ap()
    y_d = nc.dram_tensor("y", (N, HIDDEN), f32, kind="ExternalOutput").ap()

    with tile.TileContext(nc) as tc:
        with tc.tile_pool(name="sb", bufs=1) as sb, \
             tc.tile_pool(name="ps", bufs=1, space="PSUM") as ps:
            # ---- persistent SBUF tiles ----
            xt_sb = [sb.tile([128, N], bf16, tag=f"xt{c}", name=f"xt{c}")
                     for c in range(4)]
            wq_sb = sb.tile([128, 4, G_DIM], bf16, tag="wq", name="wq_sb")
            wk_sb = sb.tile([128, 4, G_DIM], bf16, tag="wk", name="wk_sb")
            wv_sb = sb.tile([128, 4, G_DIM], bf16, tag="wv", name="wv_sb")
            wo_sb = sb.tile([128, 2, HIDDEN], bf16, tag="wo", name="wo_sb")
            qb_sb = sb.tile([128, 2], f32, tag="qb", name="qb_sb")
            kb_sb = sb.tile([128, 2], f32, tag="kb", name="kb_sb")
            mk_sb = sb.tile([128, NKB], f32, tag="mk", name="mk_sb")
            # Q^T/K^T per (head-pair hp, token chunk tc): [128, 1024] bf16
            qt_sb = [[sb.tile([128, QC], bf16, tag=f"qt{hp}{t}",
                              name=f"qt{hp}{t}") for t in range(2)]
                     for hp in range(2)]
            kt_sb = [[sb.tile([128, QC], bf16, tag=f"kt{hp}{t}",
                              name=f"kt{hp}{t}") for t in range(2)]
                     for hp in range(2)]
            v_sb = sb.tile([128, NKB, G_HEADS, HEAD_DIM + 1], bf16, tag="v",
                           name="v_sb")
            e_sb = sb.tile([128, QC], f32, tag="e", name="e_sb")
            # P tiles per (head-parity, kb): [128 keys, 1024 q] bf16
            p_sb = [[sb.tile([128, QC], bf16, tag=f"p{hp}_{kb}",
                             name=f"p{hp}_{kb}") for kb in range(NKB)]
                    for hp in range(2)]
            stg_sb = [sb.tile([128, QC], f32, tag=f"stg{j}", name=f"stg{j}")
                      for j in range(2)]
            o2_sb = [sb.tile([128, 8, 128], bf16, tag=f"o2{c}", name=f"o2{c}")
                     for c in range(2)]
            ot_sb = sb.tile([128, 2, N], bf16, tag="ot", name="ot_sb")
            rc_sb = [sb.tile([128, 1], f32, tag=f"rc{j}", name=f"rc{j}")
                     for j in range(2)]
            yt_sb = [sb.tile([128, HIDDEN], f32, tag=f"yt{j}", name=f"yt{j}")
                     for j in range(2)]
            id_sb = sb.tile([128, 128], bf16, tag="id", name="id_sb")

            # ---- input DMAs ----
            warm = sb.tile([1, 4], f32, tag="warm", name="warm")
            nc.vector.memset(warm[:], 0.0)
            nc.scalar.activation(warm[:], warm[:], Exp)  # preload Exp table
            # DMA order follows first-need: K00 (wk + xt low halves), Q00
            # (wq), projection biases, then everything slot 0+ touches.
            nc.sync.dma_start(wk_sb[:], wk_d[:])
            for c in range(4):
                nc.sync.dma_start(xt_sb[c][:, 0:QC],
                                  xt_d[c * 128:(c + 1) * 128, 0:QC])
            nc.sync.dma_start(wq_sb[:], wq_d[:])
            nc.sync.dma_start(kb_sb[:], kb_d[:])
            nc.sync.dma_start(qb_sb[:], qb_d[:])
            for c in range(4):
                nc.sync.dma_start(xt_sb[c][:, QC:N],
                                  xt_d[c * 128:(c + 1) * 128, QC:N])
            nc.sync.dma_start(mk_sb[:], mk_d[:])
            nc.sync.dma_start(wv_sb[:], wv_d[:])
            nc.sync.dma_start(id_sb[:], id_d[:])
            nc.sync.dma_start(wo_sb[:], wo_d[:])
            nc.vector.memset(e_sb[:], float(np.e))
            nc.gpsimd.memset(v_sb[:], 1.0)   # ones columns default

            stag = [0]  # rotating psum tag index for s-tags

            def emit_qk(which, hp, tcn):
                """Q/K psum [128, 1024] for head-pair hp, token chunk tcn;
                DVE (psum + bias_hp) * (1/sqrt 8) -> bf16 tile."""
                w_sb, b_sb, dst = ((wq_sb, qb_sb, qt_sb) if which == "q"
                                   else (wk_sb, kb_sb, kt_sb))
                pst = ps.tile([128, QC], f32, tag=f"s{stag[0] % 3}",
                              name=f"ps_{which}{hp}{tcn}")
                stag[0] += 1
                for c in range(4):
                    for s in range(2):
                        fr = tcn * QC + s * 512
                        nc.tensor.matmul(
                            pst[:, s * 512:(s + 1) * 512],
                            lhsT=w_sb[:, c, hp * 128:(hp + 1) * 128],
                            rhs=xt_sb[c][:, fr:fr + 512],
                            start=(c == 0), stop=(c == 3))
                nc.vector.tensor_scalar(
                    dst[hp][tcn][:], pst[:], b_sb[:, hp:hp + 1], SQ, Add, Mult)

            def emit_v(kb):
                """V psum [128, 256] for key-block kb; one masked DVE copy
                into v_sb[:, kb, :, 0:64] (strided 65-per-head)."""
                psv = ps.tile([128, G_DIM], f32, tag=f"pv{kb % 2}",
                              name=f"ps_v{kb}")
                for c in range(4):
                    nc.tensor.matmul(
                        psv[:],
                        lhsT=xt_sb[c][:, kb * 128:(kb + 1) * 128],
                        rhs=wv_sb[:, c, :],
                        start=(c == 0), stop=(c == 3))
                nc.vector.tensor_scalar(
                    v_sb[:, kb, :, 0:HEAD_DIM],
                    psv.rearrange("p (h d) -> p h d", h=G_HEADS),
                    mk_sb[:, kb:kb + 1], None, Mult)

            def emit_scores_kb(qc, h, kb):
                """Score psum [128 keys, 1024 q] + exp/pow -> p_sb."""
                hp = h % 2           # ping-pong parity for p tiles
                pair = h // 2        # head-pair index into qt/kt tiles
                r0 = (h % 2) * 64    # within-pair row offset
                S = ps.tile([128, QC], f32, tag=f"s{stag[0] % 3}",
                            name=f"ps_s{qc}{h}{kb}")
                stag[0] += 1
                for w in range(2):
                    nc.tensor.matmul(
                        S[:, w * 512:(w + 1) * 512],
                        lhsT=kt_sb[pair][kb // 8][r0:r0 + 64,
                                                  (kb % 8) * 128:
                                                  (kb % 8 + 1) * 128],
                        rhs=qt_sb[pair][qc][r0:r0 + 64,
                                            w * 512:(w + 1) * 512],
                        start=True, stop=True)
                dst = p_sb[hp][kb][:]
                if kb in POOL_KBS:
                    stg = stg_sb[kb % 2]
                    nc.vector.tensor_copy(stg[:], S[:])
                    nc.gpsimd.tensor_tensor(dst, e_sb[:], stg[:], Pow)
                else:
                    nc.scalar.activation(dst, S[:], Exp)

            def emit_pv_qb(qc, h, qb, tag=None):
                """O-form PV for one 128-query block + normalize into o2."""
                hp = h % 2
                O = ps.tile([128, HEAD_DIM + 1], f32, tag=tag or f"pv{qb % 2}",
                            name=f"ps_o{qc}{h}{qb}")
                for kb in range(NKB):
                    nc.tensor.matmul(
                        O[:],
                        lhsT=p_sb[hp][kb][:, qb * 128:(qb + 1) * 128],
                        rhs=v_sb[:, kb, h, :],
                        start=(kb == 0), stop=(kb == NKB - 1))
                rc = rc_sb[qb % 2]
                nc.vector.reciprocal(rc[:], O[:, 64:65])
                nc.vector.tensor_scalar(
                    o2_sb[h // 2][:, qb, hp * 64:hp * 64 + 64],
                    O[:, 0:HEAD_DIM], rc[:, 0:1], None, Mult)

            def emit_transpose(qc, c, qb, tag=None):
                tp = ps.tile([128, 128], bf16, tag=tag or f"pv{qb % 2}",
                             name=f"ps_t{qc}{c}{qb}")
                nc.tensor.transpose(tp[:], o2_sb[c][:, qb, :], id_sb[:])
                nc.vector.tensor_copy(
                    ot_sb[:, c, qc * QC + qb * 128:qc * QC + (qb + 1) * 128],
                    tp[:])

            def emit_out_proj_qb(qc, qb):
                q0 = qc * QC + qb * 128
                Y = ps.tile([128, HIDDEN], f32, tag=f"pv{qb % 2}",
                            name=f"ps_y{qc}{qb}")
                for c in range(2):
                    nc.tensor.matmul(
                        Y[:],
                        lhsT=ot_sb[:, c, q0:q0 + 128],
                        rhs=wo_sb[:, c, :],
                        start=(c == 0), stop=(c == 1))
                yt = yt_sb[qb % 2]
                if qc == 1:
                    # tail: ACT is exp-idle by now, DVE is the tail pacer
                    nc.scalar.activation(yt[:], Y[:], Copy)
                else:
                    nc.vector.tensor_copy(yt[:], Y[:])
                nc.sync.dma_start(y_d[q0:q0 + 128, :], yt[:])

            # ---- PE warm-up: dep-free junk transposes keep the PE busy
            # through the input-DMA wait so K00 starts at full p-state
            # (cold matmuls run 2-4x slower until ~3us of continuous
            # execution). Results are garbage and never read. ----
            jk_sb = sb.tile([128, 128], bf16, tag="jk", name="jk_sb")
            nc.vector.memset(jk_sb[:], 0.0)
            for j in range(32):
                jps = ps.tile([128, 128], bf16, tag=f"pv{j % 2}",
                              name=f"ps_j{j}")
                nc.tensor.transpose(jps[:], jk_sb[:], jk_sb[:])

            # ---- lead-in: just h0's Q/K. V-projection is interleaved into
            # slots 0-1 (PV of h0 first reads V in slot 1). ----
            emit_qk("k", 0, 0)
            emit_qk("q", 0, 0)

            # deferred projection work, interleaved into the attention
            # stream at (slot, kb) chosen to even out PE load per slot
            # while meeting need-bys: K01 by s0kb8; K10/K11 by s2 scores
            # lhsT; Q10 by s2kb0 rhs; Q01 by s4; Q11 by s6.
            deferred = {
                (0, 3): lambda: emit_qk("k", 0, 1),
                (1, 3): lambda: emit_qk("k", 1, 0),
                (1, 11): lambda: emit_qk("q", 1, 0),
                (2, 3): lambda: emit_qk("k", 1, 1),
                (2, 11): lambda: emit_qk("q", 0, 1),
                (3, 11): lambda: emit_qk("q", 1, 1),
            }
            vq = list(range(NKB))   # V-projection kbs, drained slots 0-1
            post = []   # pending per-head PV/transpose/out-proj closures

            def emit_slot(qc, h, si):
                """One head-slot: scores+exp for (qc, h) interleaved with
                pending PV of the previous head, V-projection (slots 0-1)
                and deferred projections. Posts pop at kb>=3 so the
                previous slot's last exps (incl. pool-path latency) are
                done before the in-order PE hits the PV matmuls that read
                them. post MUST fully drain within the slot: the p tiles
                it reads are double-buffered and rewritten next slot."""
                for kb in range(NKB):
                    emit_scores_kb(qc, h, kb)
                    if vq and (si == 0 and kb < 14 or si == 1 and kb < 2):
                        emit_v(vq.pop(0))
                    if kb >= 3 and post:
                        post.pop(0)()
                    d = deferred.pop((si, kb), None)
                    if d:
                        d()
                while post:
                    post.pop(0)()

            def make_post(qc, h, qb, tail=False):
                # in the tail (after the last score slot) the s-banks are
                # free: spread PV/transpose psums over them so the per-qb
                # finalize chains aren't serialized on two pv tags.
                ptag = f"s{qb % 3}" if tail else None
                def f():
                    emit_pv_qb(qc, h, qb, tag=ptag)
                    if h % 2 == 1:
                        emit_transpose(qc, h // 2, qb, tag=ptag)
                    if h == 3:
                        emit_out_proj_qb(qc, qb)
                return f

            slots = [(qc, h) for qc in range(2) for h in range(G_HEADS)]
            for i, (qc, h) in enumerate(slots):
                emit_slot(qc, h, i)
                # queue this head's PV (+ per-qb transpose/out-proj chain)
                for qb in range(8):
                    post.append(make_post(qc, h, qb, tail=(i == 7)))

            # drain everything still pending
            while post:
                post.pop(0)()
            while deferred:
                deferred.pop(0)()

    nc.compile()
    return nc


def _get_program():
    if "nc" not in _CACHE:
        _CACHE["nc"] = _build_program()
    return _CACHE["nc"]


def _prep_inputs(x, cancer_type, attn_mask, wq, bq, wk, bk, wv, bv, wo, bo,
                 bias_emb, keymod_emb):
    """Host-side shard prep: returns (in_maps list of 8, epilogue (512,))."""
    x = np.asarray(x, dtype=np.float32)
    ct = np.asarray(cancer_type).astype(np.int64)
    mask = np.asarray(attn_mask)
    wq = np.asarray(wq, dtype=np.float32)
    wk = np.asarray(wk, dtype=np.float32)
    wv = np.asarray(wv, dtype=np.float32)
    wo = np.asarray(wo, dtype=np.float32)
    bq = np.asarray(bq, dtype=np.float32)
    bk = np.asarray(bk, dtype=np.float32)
    bv = np.asarray(bv, dtype=np.float32)
    bo = np.asarray(bo, dtype=np.float32)
    keymod = np.asarray(keymod_emb, dtype=np.float32)

    wqt = np.ascontiguousarray(wq.T).astype(BF16)     # (in 512, out 512)
    wkt = np.ascontiguousarray(wk.T).astype(BF16)
    wvt = np.ascontiguousarray(wv.T).astype(BF16)
    wot = np.ascontiguousarray(wo.T).astype(BF16)

    xt_all = [np.ascontiguousarray(x[b].T).astype(BF16) for b in range(B)]
    mks = [np.where(mask[b], np.float32(0.0), np.float32(1.0))
           .reshape(NKB, 128).T.copy() for b in range(B)]

    in_maps = []
    for core in range(N_CORES):
        b, g = core // 2, core % 2
        gs = slice(g * G_DIM, (g + 1) * G_DIM)
        qbias = np.ascontiguousarray(
            (bq[gs] * SQ).reshape(2, 128).T).astype(np.float32)
        kfull = (bk + keymod[ct[b]])[gs] * SQ
        kbias = np.ascontiguousarray(kfull.reshape(2, 128).T).astype(np.float32)
        in_maps.append({
            "id": np.eye(128, dtype=BF16),
            "xt": xt_all[b],
            "wq": np.ascontiguousarray(
                wqt[:, gs].reshape(4, 128, G_DIM).transpose(1, 0, 2)),
            "wk": np.ascontiguousarray(
                wkt[:, gs].reshape(4, 128, G_DIM).transpose(1, 0, 2)),
            "wv": np.ascontiguousarray(
                wvt[:, gs].reshape(4, 128, G_DIM).transpose(1, 0, 2)),
            "wo": np.ascontiguousarray(
                wot[gs, :].reshape(2, 128, HIDDEN).transpose(1, 0, 2)),
            "qb": qbias,
            "kb": kbias,
            "mk": mks[b],
        })
    epilogue = (bv @ wo.T + bo).astype(np.float32)    # (512,)
    return in_maps, epilogue


def kernel(**inputs):
    from concourse import bass_utils

    nc = _get_program()
    in_maps, epilogue = _prep_inputs(**inputs)
    res = bass_utils.run_bass_kernel_spmd(nc, in_maps,
                                          core_ids=list(range(N_CORES)))
    out = np.empty((B, N, HIDDEN), dtype=np.float32)
    for b in range(B):
        out[b] = res.results[2 * b]["y"] + res.results[2 * b + 1]["y"] + epilogue
    return out

